# revision 13
# baseline (speedup 1.0000x reference)
"""Trainium2 Bass kernel for nn_BrainBottleneckLocal (dense_cnn).

Sharding: spatial rows. H=16 rows are split 2-per-core across 8 NeuronCores.
Every layer is then core-local:
  - conv1x1 #1 (+BN1+ReLU) is computed on the core's 2 rows plus a 1-row halo
    on each side (4 rows total, boundary rows zero-padded by the host).
  - the locally-connected 3x3 layer (per-location weights) needs exactly that
    halo; lc_w (604 MB fp32) is split 8x by row so each core only loads its
    own 32 locations (18.9 MB as fp8 e4m3, pre-scaled x256 to stay in fp8's
    normal range; the 1/256 is folded into the BN2 activation scale).
  - conv1x1 #2 (+BN3), residual add + ReLU, and the opponent-inhibition
    channel mixing are all per-location ops on the core's own 2 rows.

Precision: fp16 trunk (conv1 weights+input, LC patches, out2, conv3) so only
three cheap fp8 quantizations remain: the LC weight stream (the DMA-dominant
tensor), and the inhibition matmul's two operands (g matrix + relu'd
activations), whose error is diluted by the 1/(1+inh) form. The inhibition
matmul runs fp8 DoubleRow (2x PE rate); the LC matmul runs normal mode with
fp16 patches stationary and the fp8 weight stream moving, two locations
concurrently in different PE column groups (tile_position col-tiling).

The residual identity is the same fp16 tensor as the conv1 input (host sends
x + beta3; conv1's bias is corrected by -W1'@beta3 so conv1 still sees x).
The final output is stored fp16 and upcast on the host. Free-dim order is
(h, w, n) everywhere; LC's [n, o] psum is PE-transposed back to [o, n] in
batched [128,128] transposes (2 locations per transpose).
"""

import math
from contextlib import ExitStack

import numpy as np

import concourse.bacc as bacc
import concourse.bass as bass
import concourse.mybir as mybir
import concourse.tile as tile
from concourse.bass_utils import run_bass_kernel_spmd

F32 = mybir.dt.float32
FP16 = mybir.dt.float16
FP8 = mybir.dt.float8e4
NPF16 = np.float16
NPF8 = mybir.dt.np(FP8)

EPS = 1e-5
N, CIN, H, W = 64, 1024, 16, 16
WID, COUT = 256, 1024
NCORES = 8
RPC = H // NCORES          # rows per core = 2
HLO = RPC + 2              # rows incl halo = 4
WP = W + 2                 # padded width = 18
NLOC = RPC * W             # LC locations per core = 32
CC1 = CIN // 128           # 8
CCW = WID // 128           # 2
CC3 = COUT // 128          # 8
FR = RPC * W * N           # free size of per-core row block = 2048, (h,w,n)
SW = 256.0                 # host pre-scale on LC weights (fp8 range)
ISW = 1.0 / SW
KF = 6                     # inhibition: Fourier modes kept (cos 0..KF, sin)
J = 3                      # inhibition: Taylor orders in sigma
R = 2 * KF + 1
JR = 64                    # low-rank inhibition rank (39 used, zero-padded
                           # to a native PE tile size)
AF = mybir.ActivationFunctionType
ALU = mybir.AluOpType
DR = mybir.MatmulPerfMode.DoubleRow


def _declare_drams(nc):
    ap = {}
    ap["xh"] = nc.dram_tensor("xh", [CC1, 128, RPC * W * N], FP16,
                              kind="ExternalInput").ap()
    ap["xq"] = nc.dram_tensor("xq", [CC1, 128, 2 * W * N], FP8,
                              kind="ExternalInput").ap()
    ap["lcw"] = nc.dram_tensor("lcw", [NLOC, 128, 9 * CCW * WID], FP8,
                               kind="ExternalInput").ap()
    ap["w1t"] = nc.dram_tensor("w1t", [128, CC1, WID], FP16,
                               kind="ExternalInput").ap()
    ap["w3t"] = nc.dram_tensor("w3t", [128, CCW, COUT], FP16,
                               kind="ExternalInput").ap()
    ap["g1t"] = nc.dram_tensor("g1t", [128, CC3, JR], FP16,
                               kind="ExternalInput").ap()
    ap["f2t"] = nc.dram_tensor("f2t", [JR, CC3, 128], FP16,
                               kind="ExternalInput").ap()
    ap["b1"] = nc.dram_tensor("b1", [CCW, 128, 1], F32,
                              kind="ExternalInput").ap()
    ap["b2"] = nc.dram_tensor("b2", [CCW, 128, 1], F32,
                              kind="ExternalInput").ap()
    ap["ident"] = nc.dram_tensor("ident", [128, 128], FP16,
                                 kind="ExternalInput").ap()
    ap["out"] = nc.dram_tensor("out", [CC3, 128, FR], FP16,
                               kind="ExternalOutput").ap()
    return ap


def _build_nc(ktimes: int = 1):
    nc = bacc.Bacc("TRN2", target_bir_lowering=False, debug=False,
                   num_devices=NCORES)
    ap = _declare_drams(nc)
    with tile.TileContext(nc) as tc:
        if ktimes == 1:
            _trace_kernel(tc, nc, ap)
        else:
            # hardware loop for timing runs: one dispatch, ktimes execs
            with tc.For_i(0, ktimes, 1):
                _trace_kernel(tc, nc, ap)
    nc.compile()
    return nc


def _trace_kernel(tc, nc, ap):
    with ExitStack() as ctx:
        persist = ctx.enter_context(tc.tile_pool(name="persist", bufs=1))
        psum = ctx.enter_context(
            tc.tile_pool(name="psum", bufs=3, space="PSUM"))

        # ---- persistent constants (scalar DGE queue) -------------------
        w1_t = persist.tile([128, CC1, WID], FP16, name="w1t", tag="w1t")
        nc.scalar.dma_start(out=w1_t, in_=ap["w1t"])
        w3_t = persist.tile([128, CCW, COUT], FP16, name="w3t", tag="w3t")
        nc.scalar.dma_start(out=w3_t, in_=ap["w3t"])
        g1_t = persist.tile([128, CC3, JR], FP16, name="g1t", tag="g1t")
        nc.scalar.dma_start(out=g1_t, in_=ap["g1t"])
        f2_t = persist.tile([JR, CC3, 128], FP16, name="f2t", tag="f2t")
        nc.scalar.dma_start(out=f2_t, in_=ap["f2t"])
        ident_t = persist.tile([128, 128], FP16, name="ident", tag="ident")
        nc.scalar.dma_start(out=ident_t, in_=ap["ident"])

        def load_bias(name, nch):
            outl = []
            for c in range(nch):
                t = persist.tile([128, 1], F32, name=f"{name}_{c}",
                                 tag=f"{name}{c}")
                nc.scalar.dma_start(out=t, in_=ap[name][c])
                outl.append(t)
            return outl

        b1_t = load_bias("b1", CCW)
        b2_t = load_bias("b2", CCW)

        # x (+beta3): own rows fp16 (conv1 moving operand AND residual
        # identity); the two halo rows fp8 (feed conv1 -> LC taps only)
        xh_t = persist.tile([128, CC1, RPC, W, N], FP16, name="xh",
                            tag="xh")
        xq_t = persist.tile([128, CC1, 2, W, N], FP8, name="xq", tag="xq")
        for cc in range(CC1):
            nc.sync.dma_start(out=xh_t[:, cc], in_=ap["xh"][cc])
            nc.sync.dma_start(out=xq_t[:, cc], in_=ap["xq"][cc])

        out2_t = persist.tile([128, CCW, NLOC * N], FP16, name="out2",
                              tag="out2")
        resb_t = persist.tile([128, CC3, FR], FP8, name="resb", tag="resb")

        # ---- PE warm-up: keep HAM busy while xh streams in -------------
        wu_t = persist.tile([128, 512], FP16, name="wu", tag="wu")
        nc.gpsimd.memset(wu_t, 0.25)
        for _ in range(16):
            pw = psum.tile([128, 512], F32, name="pw", tag="a", bufs=3)
            nc.tensor.matmul(pw, wu_t[:, 0:128], wu_t, start=True, stop=True)

        # out1 padded: [p, h4, wp18, ch2, n64] fp16, zeroed W-pad columns
        out1p_pool = ctx.enter_context(tc.tile_pool(name="o1p", bufs=1))
        out1p = out1p_pool.tile([128, HLO, WP, CCW, N], FP16, name="out1p",
                                tag="o1p")
        nc.gpsimd.memset(out1p, 0.0)

        # ---- phase 1: conv1x1 #1 + BN1 + ReLU on 4 halo rows -----------
        for h in range(HLO):
            for oc in range(CCW):
                for ns in range(2):
                    ps = psum.tile([128, 512], F32, name="ps1", tag="a",
                                   bufs=3)
                    for cc in range(CC1):
                        if h == 0:
                            mv = xq_t[:, cc, 0, ns * 8:(ns + 1) * 8, :]
                        elif h == HLO - 1:
                            mv = xq_t[:, cc, 1, ns * 8:(ns + 1) * 8, :]
                        else:
                            mv = xh_t[:, cc, h - 1, ns * 8:(ns + 1) * 8, :]
                        nc.tensor.matmul(
                            ps,
                            w1_t[:, cc, oc * 128:(oc + 1) * 128],
                            mv,
                            start=(cc == 0), stop=(cc == CC1 - 1))
                    nc.scalar.activation(
                        out=out1p[:, h, 1 + ns * 8:1 + (ns + 1) * 8, oc, :],
                        in_=ps, func=AF.Relu, bias=b1_t[oc], scale=1.0)

        # pools for LC and later phases
        lcw_pool = ctx.enter_context(tc.tile_pool(name="lcwp", bufs=8))
        tmp_pool = ctx.enter_context(tc.tile_pool(name="tmpp", bufs=4))
        t_pool = ctx.enter_context(tc.tile_pool(name="tp", bufs=1))
        div_pool = ctx.enter_context(tc.tile_pool(name="divp", bufs=2))

        # ---- phase 2: locally-connected 3x3 + BN2 + ReLU ---------------
        # Two locations run concurrently in different PE column groups:
        # loc A -> psum partitions 0:64 (tile_position (0,0)), loc B ->
        # 64:128 ((0,64)). Patches are stationary fp16 [128, 64]; the fp8
        # weight stream is the moving operand. psum2 [128(2 locs x n), 256]
        # is copied to fp16 and PE-transposed back to [o, (2 locs x n)].
        for grp in range(NLOC // 4):
            pst = psum.tile([128, CCW, 256], FP16, name="pst", tag="tp",
                            bufs=2)
            for pair in range(2):
                locA = grp * 4 + pair * 2
                lwAB = []
                for li in range(2):
                    lw = lcw_pool.tile([128, 9, CCW, WID], FP8,
                                       name="lcw_t", tag="lcw")
                    nc.sync.dma_start(out=lw, in_=ap["lcw"][locA + li])
                    lwAB.append(lw)
                ps2 = psum.tile([128, WID], F32, name="ps2", tag="lc",
                                bufs=2)
                for kc in range(18):
                    dk, ch = divmod(kc, 2)
                    di, dj = divmod(dk, 3)
                    for li in range(2):
                        hl, j = divmod(locA + li, W)
                        nc.tensor.matmul(
                            ps2[li * 64:(li + 1) * 64, :],
                            out1p[:, hl + di, j + dj, ch, :],
                            lwAB[li][:, dk, ch, :],
                            start=(kc == 0), stop=(kc == 17),
                            tile_position=(0, li * 64))
                tmpb = tmp_pool.tile([128, 256], FP16, name="tmpb",
                                     tag="tmpb")
                nc.vector.tensor_copy(out=tmpb, in_=ps2)
                for oc in range(CCW):
                    nc.tensor.transpose(
                        pst[:, oc, pair * 128:(pair + 1) * 128],
                        tmpb[:, oc * 128:(oc + 1) * 128], ident_t)
            for oc in range(CCW):
                nc.scalar.activation(
                    out=out2_t[:, oc, grp * 256:(grp + 1) * 256],
                    in_=pst[:, oc, :], func=AF.Relu, bias=b2_t[oc],
                    scale=ISW)

        # ---- phase 3+4 merged, per 512-slice of (h,w,n): conv1x1 #2 +
        # BN3 + residual, then inhibition + divide + store. ns-outer order
        # lets slice ns start as soon as LC has produced locations
        # 8ns..8ns+7, overlapping the remaining LC weight stream.
        for ns in range(FR // 512):
            sl = slice(ns * 512, (ns + 1) * 512)
            hl, nw = divmod(ns, 2)
            tts = []
            for oc3 in range(CC3):
                ps = psum.tile([128, 512], F32, name="ps3", tag="a", bufs=3)
                for oc in range(CCW):
                    nc.tensor.matmul(
                        ps, w3_t[:, oc, oc3 * 128:(oc3 + 1) * 128],
                        out2_t[:, oc, sl],
                        start=(oc == 0), stop=(oc == CCW - 1))
                tt = t_pool.tile([128, 512], F32, name=f"tt{oc3}",
                                 tag=f"tt{oc3}")
                # t = conv3*inv3 + beta3 + x  (beta3 lives in xh)
                nc.vector.scalar_tensor_tensor(
                    out=tt, in0=ps, scalar=0.0,
                    in1=xh_t[:, oc3, hl, nw * 8:(nw + 1) * 8, :],
                    op0=ALU.add, op1=ALU.add)
                nc.scalar.activation(out=resb_t[:, oc3, sl], in_=tt,
                                     func=AF.Relu)
                tts.append(tt)
            # inhibition, low-rank: inh = F2^T (G1^T relu(t));  G1/F2 fold
            # the Gaussian mixing matrix's Fourier x Taylor factorization
            zp = psum.tile([JR, 512], F32, name="zp", tag="z", bufs=1)
            for cc in range(CC3):
                nc.tensor.matmul(zp, g1_t[:, cc, :], resb_t[:, cc, sl],
                                 start=(cc == 0), stop=(cc == CC3 - 1))
            zs = div_pool.tile([JR, 512], FP16, name="zs", tag="zs")
            nc.vector.tensor_copy(out=zs, in_=zp)
            for oc in range(CC3):
                ps = psum.tile([128, 512], F32, name="ps4", tag="a", bufs=3)
                nc.tensor.matmul(ps, f2_t[:, oc, :], zs,
                                 start=True, stop=True)
                den = div_pool.tile([128, 512], F32, name="den", tag="den")
                nc.scalar.activation(out=den, in_=ps, func=AF.Copy,
                                     scale=1.0, bias=1.0)
                rec = div_pool.tile([128, 512], F32, name="rec", tag="rec")
                nc.vector.reciprocal_approx_fast(out=rec, in_=den)
                fin = div_pool.tile([128, 512], FP16, name="fin", tag="fin")
                # final = max(t, 0) * 1/(1+inh)   (recip > 0 always)
                nc.vector.scalar_tensor_tensor(
                    out=fin, in0=tts[oc], scalar=0.0, in1=rec,
                    op0=ALU.max, op1=ALU.mult)
                nc.scalar.dma_start(out=ap["out"][oc][:, sl], in_=fin)


def _prep_inputs(x, w1, g1, b1, m1, v1, lc_w, g2, b2, m2, v2,
                 w3, g3, b3, m3, v3, sigmas):
    """Host-side shard + layout prep. Returns per-core input maps."""
    f4 = np.float32
    x = np.asarray(x, f4)
    inv1 = (g1 / np.sqrt(v1 + EPS)).astype(f4)
    beta1 = (b1 - m1 * inv1).astype(f4)
    inv2 = (g2 / np.sqrt(v2 + EPS)).astype(f4)
    beta2 = (b2 - m2 * inv2).astype(f4)
    inv3 = (g3 / np.sqrt(v3 + EPS)).astype(f4)
    beta3 = (b3 - m3 * inv3).astype(f4)

    def to8(a):
        return np.clip(a, -240.0, 240.0).astype(NPF8)

    # conv1 weight [p, cc, o] with c = cc*128 + p; bias corrected for the
    # beta3 folded into xh (conv1 must see x, not x + beta3)
    w1s = np.asarray(w1, f4) * inv1[:, None]                   # (WID, CIN)
    b1p = beta1 - w1s @ beta3
    w1t = np.ascontiguousarray(
        w1s.T.reshape(CC1, 128, WID).transpose(1, 0, 2)).astype(NPF16)
    # w3t [p, oc, o3]
    w3s = (np.asarray(w3, f4) * inv3[:, None]).T               # (WID, COUT)
    w3t = np.ascontiguousarray(
        w3s.reshape(CCW, 128, COUT).transpose(1, 0, 2)).astype(NPF16)

    # inhibition mixing matrix g[o,c] = E(d_oc; sig_c)/s(sig_c) is a
    # near-circulant Gaussian (the 1/(2.5066 sig) prefactor cancels in the
    # column normalization). Factor it as Fourier (cos series in (c-o),
    # KF modes) x Taylor (J orders in sig_c around mean sig):
    #   inh = F2^T (G1^T y),  G1[c,(j,r)] = basis_r(c) dlt_c^j/(j! s_c),
    #   F2[(j,r),o] = a_{j,k(r)} basis_r(o)
    sig = np.maximum(np.asarray(sigmas, np.float64), 0.5)
    sig0 = float(sig.mean())
    dlt = sig - sig0
    mm = np.arange(COUT)
    dm = np.abs(mm - COUT // 2).astype(np.float64)
    s = np.exp(-dm[:, None] ** 2 / (2.0 * sig[None, :] ** 2)).sum(0)
    E0 = np.exp(-dm ** 2 / (2 * sig0 ** 2))
    E1 = (dm ** 2 / sig0 ** 3) * E0
    E2 = (dm ** 4 / sig0 ** 6 - 3 * dm ** 2 / sig0 ** 4) * E0
    acoef = np.zeros((J, KF + 1))
    for j, hker in enumerate([E0, E1, E2][:J]):
        Fc = np.fft.rfft(hker).real / COUT
        a = 2.0 * Fc[:KF + 1]
        a[0] = Fc[0]
        acoef[j] = a
    ang = 2 * np.pi * mm[:, None] * np.arange(KF + 1)[None, :] / COUT
    cosb, sinb = np.cos(ang), np.sin(ang)
    basis = np.concatenate([cosb, sinb[:, 1:]], 1)             # (C, R)
    kmap = np.concatenate([np.arange(KF + 1), np.arange(1, KF + 1)])
    fact = [1.0, 1.0, 2.0, 6.0]
    G1 = np.zeros((COUT, JR))
    F2 = np.zeros((JR, COUT))
    assert J * R <= JR
    for j in range(J):
        for r in range(R):
            G1[:, j * R + r] = basis[:, r] * (dlt ** j) / (fact[j] * s)
            F2[j * R + r, :] = acoef[j, kmap[r]] * basis[:, r]
    g1t = np.ascontiguousarray(
        G1.reshape(CC3, 128, JR).transpose(1, 0, 2)).astype(NPF16)
    f2t = np.ascontiguousarray(F2.reshape(JR, CC3, 128)).astype(NPF16)

    # x (+beta3): (C, Hpad, W, N), rows zero-padded at both ends; own
    # rows shipped fp16, halo rows fp8
    xt = x.transpose(1, 2, 3, 0)                               # (C, H, W, N)
    xpad = np.zeros((CIN, H + 2, W, N), f4)
    xpad[:, 1:H + 1] = xt
    xpb = xpad + beta3[:, None, None, None]
    xh = xpb.astype(NPF16)
    xq8 = to8(xpb)

    # lc_w: (1,O,C,H,W,9) -> (H, W, p, dk, ch, o), scaled x256, fp8
    a = np.asarray(lc_w[0], f4) * (inv2[:, None, None, None, None] * SW)
    a = a.transpose(2, 3, 1, 4, 0)                 # (H, W, C, 9, O)
    a = a.reshape(H, W, CCW, 128, 9, WID).transpose(0, 1, 3, 4, 2, 5)
    lcw8 = to8(np.ascontiguousarray(a))            # (H, W, 128, 9, 2, WID)

    com = {
        "ident": np.eye(128, dtype=NPF16),
        "w1t": w1t, "w3t": w3t, "g1t": g1t, "f2t": f2t,
        "b1": b1p.reshape(CCW, 128, 1), "b2": beta2.reshape(CCW, 128, 1),
    }
    in_maps = []
    for r in range(NCORES):
        r0 = r * RPC
        xb = np.ascontiguousarray(xh[:, r0 + 1:r0 + 1 + RPC]).reshape(
            CC1, 128, RPC * W * N)
        xq = np.ascontiguousarray(xq8[:, [r0, r0 + HLO - 1]]).reshape(
            CC1, 128, 2 * W * N)
        lw = np.ascontiguousarray(lcw8[r0:r0 + RPC]).reshape(
            NLOC, 128, 9 * CCW * WID)
        if r == 0 or r == NCORES - 1:
            lw = lw.copy()
            lwv = lw.reshape(NLOC, 128, 9, CCW, WID)
            if r == 0:           # row 0 locations: di=0 taps read row -1
                lwv[0:W, :, 0:3] = 0
            if r == NCORES - 1:  # row 15 locations: di=2 taps read row 16
                lwv[W:2 * W, :, 6:9] = 0
        in_maps.append(dict(com, xh=xb, xq=xq, lcw=lw))
    return in_maps


def _assemble(results):
    """results: per-core dicts with 'out' [CC3,128,FR] -> (N,C,H,W) fp32"""
    full = np.empty((N, COUT, H, W), np.float32)
    for r, res in enumerate(results):
        o = res["out"].astype(np.float32).reshape(CC3, 128, RPC, W, N)
        # (cc, p, hl, j, n) -> (n, c, h, w)
        o = o.transpose(4, 0, 1, 2, 3).reshape(N, COUT, RPC, W)
        full[:, :, r * RPC:(r + 1) * RPC, :] = o
    return full


_NC_CACHE = {}


def get_nc(ktimes: int = 1):
    if ktimes not in _NC_CACHE:
        _NC_CACHE[ktimes] = _build_nc(ktimes)
    return _NC_CACHE[ktimes]


def kernel(**inputs):
    nc = get_nc()
    in_maps = _prep_inputs(**inputs)
    res = run_bass_kernel_spmd(nc, in_maps, core_ids=list(range(NCORES)))
    return _assemble(res.results)


if __name__ == "__main__":
    rng = np.random.default_rng(0)
    ins = {
        "x": rng.standard_normal((N, CIN, H, W), np.float32),
        "w1": (rng.standard_normal((WID, CIN), np.float32) * 0.05),
        "g1": rng.random(WID, np.float32),
        "b1": rng.standard_normal(WID, np.float32) * 0.05,
        "m1": np.zeros(WID, np.float32),
        "v1": np.ones(WID, np.float32),
        "lc_w": rng.standard_normal((1, WID, WID, H, W, 9),
                                    np.float32) * 0.05,
        "g2": rng.random(WID, np.float32),
        "b2": rng.standard_normal(WID, np.float32) * 0.05,
        "m2": np.zeros(WID, np.float32),
        "v2": np.ones(WID, np.float32),
        "w3": rng.standard_normal((COUT, WID), np.float32) * 0.05,
        "g3": rng.random(COUT, np.float32),
        "b3": rng.standard_normal(COUT, np.float32) * 0.05,
        "m3": np.zeros(COUT, np.float32),
        "v3": np.ones(COUT, np.float32),
        "sigmas": rng.random(COUT, np.float32) + COUT / 8.0,
    }
    out = kernel(**ins)
    print("out", out.shape, out.dtype, float(np.abs(out).max()))


# revision 14
# speedup vs baseline: 1.4487x; 1.4487x over previous
"""Trainium2 Bass kernel for nn_BrainBottleneckLocal (dense_cnn).

Sharding: spatial rows. H=16 rows are split 2-per-core across 8 NeuronCores.
Every layer is then core-local:
  - conv1x1 #1 (+BN1+ReLU) is computed on the core's 2 rows plus a 1-row halo
    on each side (4 rows total, boundary rows zero-padded by the host).
  - the locally-connected 3x3 layer (per-location weights) needs exactly that
    halo; lc_w (604 MB fp32) is split 8x by row so each core only loads its
    own 32 locations (18.9 MB as fp8 e4m3, pre-scaled x256 to stay in fp8's
    normal range; the 1/256 is folded into the BN2 activation scale).
  - conv1x1 #2 (+BN3), residual add + ReLU, and the opponent-inhibition
    channel mixing are all per-location ops on the core's own 2 rows.

Precision: fp16 trunk (conv1 weights+input, LC patches, out2, conv3) so only
three cheap fp8 quantizations remain: the LC weight stream (the DMA-dominant
tensor), and the inhibition matmul's two operands (g matrix + relu'd
activations), whose error is diluted by the 1/(1+inh) form. The inhibition
matmul runs fp8 DoubleRow (2x PE rate); the LC matmul runs normal mode with
fp16 patches stationary and the fp8 weight stream moving, two locations
concurrently in different PE column groups (tile_position col-tiling).

The residual identity is the same fp16 tensor as the conv1 input (host sends
x + beta3; conv1's bias is corrected by -W1'@beta3 so conv1 still sees x).
The final output is stored fp16 and upcast on the host. Free-dim order is
(h, w, n) everywhere; LC's [n, o] psum is PE-transposed back to [o, n] in
batched [128,128] transposes (2 locations per transpose).
"""

import math
from contextlib import ExitStack

import numpy as np

import concourse.bacc as bacc
import concourse.bass as bass
import concourse.mybir as mybir
import concourse.tile as tile
from concourse.bass_utils import run_bass_kernel_spmd

F32 = mybir.dt.float32
FP16 = mybir.dt.float16
FP8 = mybir.dt.float8e4
NPF16 = np.float16
NPF8 = mybir.dt.np(FP8)

EPS = 1e-5
N, CIN, H, W = 64, 1024, 16, 16
WID, COUT = 256, 1024
NCORES = 8
RPC = H // NCORES          # rows per core = 2
HLO = RPC + 2              # rows incl halo = 4
WP = W + 2                 # padded width = 18
NLOC = RPC * W             # LC locations per core = 32
CC1 = CIN // 128           # 8
CCW = WID // 128           # 2
CC3 = COUT // 128          # 8
FR = RPC * W * N           # free size of per-core row block = 2048, (h,w,n)
SW = 256.0                 # host pre-scale on LC weights (fp8 range)
ISW = 1.0 / SW
KF = 6                     # inhibition: Fourier modes kept (cos 0..KF, sin)
J = 3                      # inhibition: Taylor orders in sigma
R = 2 * KF + 1
JR = 64                    # low-rank inhibition rank (39 used, zero-padded
                           # to a native PE tile size)
AF = mybir.ActivationFunctionType
ALU = mybir.AluOpType
DR = mybir.MatmulPerfMode.DoubleRow


def _declare_drams(nc):
    ap = {}
    ap["xh"] = nc.dram_tensor("xh", [CC1, 128, RPC * W * N], FP16,
                              kind="ExternalInput").ap()
    ap["xq"] = nc.dram_tensor("xq", [CC1, 128, 2 * W * N], FP8,
                              kind="ExternalInput").ap()
    ap["lcw"] = nc.dram_tensor("lcw", [NLOC, 128, 9 * CCW * WID], FP8,
                               kind="ExternalInput").ap()
    ap["w1t"] = nc.dram_tensor("w1t", [128, CC1, WID], FP16,
                               kind="ExternalInput").ap()
    ap["w3t"] = nc.dram_tensor("w3t", [128, CCW, COUT], FP16,
                               kind="ExternalInput").ap()
    ap["g1t"] = nc.dram_tensor("g1t", [128, CC3, JR], FP16,
                               kind="ExternalInput").ap()
    ap["f2t"] = nc.dram_tensor("f2t", [JR, CC3, 128], FP16,
                               kind="ExternalInput").ap()
    ap["b1"] = nc.dram_tensor("b1", [CCW, 128, 1], F32,
                              kind="ExternalInput").ap()
    ap["b2"] = nc.dram_tensor("b2", [CCW, 128, 1], F32,
                              kind="ExternalInput").ap()
    ap["ident"] = nc.dram_tensor("ident", [128, 128], FP16,
                                 kind="ExternalInput").ap()
    ap["out"] = nc.dram_tensor("out", [CC3, 128, FR], FP16,
                               kind="ExternalOutput").ap()
    return ap


def _build_nc(ktimes: int = 1):
    nc = bacc.Bacc("TRN2", target_bir_lowering=False, debug=False,
                   num_devices=NCORES)
    ap = _declare_drams(nc)
    with tile.TileContext(nc) as tc:
        if ktimes == 1:
            _trace_kernel(tc, nc, ap)
        else:
            # hardware loop for timing runs: one dispatch, ktimes execs
            with tc.For_i(0, ktimes, 1):
                _trace_kernel(tc, nc, ap)
    nc.compile()
    return nc


def _trace_kernel(tc, nc, ap):
    with ExitStack() as ctx:
        persist = ctx.enter_context(tc.tile_pool(name="persist", bufs=1))
        psum = ctx.enter_context(
            tc.tile_pool(name="psum", bufs=3, space="PSUM"))

        # ---- persistent constants (scalar DGE queue) -------------------
        w1_t = persist.tile([128, CC1, WID], FP16, name="w1t", tag="w1t")
        nc.scalar.dma_start(out=w1_t, in_=ap["w1t"])
        w3_t = persist.tile([128, CCW, COUT], FP16, name="w3t", tag="w3t")
        nc.scalar.dma_start(out=w3_t, in_=ap["w3t"])
        g1_t = persist.tile([128, CC3, JR], FP16, name="g1t", tag="g1t")
        nc.scalar.dma_start(out=g1_t, in_=ap["g1t"])
        f2_t = persist.tile([JR, CC3, 128], FP16, name="f2t", tag="f2t")
        nc.scalar.dma_start(out=f2_t, in_=ap["f2t"])
        ident_t = persist.tile([128, 128], FP16, name="ident", tag="ident")
        nc.scalar.dma_start(out=ident_t, in_=ap["ident"])

        def load_bias(name, nch):
            outl = []
            for c in range(nch):
                t = persist.tile([128, 1], F32, name=f"{name}_{c}",
                                 tag=f"{name}{c}")
                nc.scalar.dma_start(out=t, in_=ap[name][c])
                outl.append(t)
            return outl

        b1_t = load_bias("b1", CCW)
        b2_t = load_bias("b2", CCW)

        # x (+beta3): own rows fp16 (conv1 moving operand AND residual
        # identity); the two halo rows fp8 (feed conv1 -> LC taps only)
        xh_t = persist.tile([128, CC1, RPC, W, N], FP16, name="xh",
                            tag="xh")
        xq_t = persist.tile([128, CC1, 2, W, N], FP8, name="xq", tag="xq")
        for cc in range(CC1):
            nc.sync.dma_start(out=xh_t[:, cc], in_=ap["xh"][cc])
            nc.sync.dma_start(out=xq_t[:, cc], in_=ap["xq"][cc])

        out2_t = persist.tile([128, CCW, NLOC * N], FP16, name="out2",
                              tag="out2")
        resb_t = persist.tile([128, CC3, FR], FP8, name="resb", tag="resb")

        # ---- PE warm-up: keep HAM busy while xh streams in -------------
        wu_t = persist.tile([128, 512], FP16, name="wu", tag="wu")
        nc.gpsimd.memset(wu_t, 0.25)
        for _ in range(16):
            pw = psum.tile([128, 512], F32, name="pw", tag="a", bufs=3)
            nc.tensor.matmul(pw, wu_t[:, 0:128], wu_t, start=True, stop=True)

        # out1 padded: [p, h4, wp18, ch2, n64] fp16, zeroed W-pad columns
        out1p_pool = ctx.enter_context(tc.tile_pool(name="o1p", bufs=1))
        out1p = out1p_pool.tile([128, HLO, WP, CCW, N], FP16, name="out1p",
                                tag="o1p")
        nc.gpsimd.memset(out1p, 0.0)

        # ---- phase 1: conv1x1 #1 + BN1 + ReLU on 4 halo rows -----------
        for h in range(HLO):
            for oc in range(CCW):
                for ns in range(2):
                    ps = psum.tile([128, 512], F32, name="ps1", tag="a",
                                   bufs=3)
                    for cc in range(CC1):
                        if h == 0:
                            mv = xq_t[:, cc, 0, ns * 8:(ns + 1) * 8, :]
                        elif h == HLO - 1:
                            mv = xq_t[:, cc, 1, ns * 8:(ns + 1) * 8, :]
                        else:
                            mv = xh_t[:, cc, h - 1, ns * 8:(ns + 1) * 8, :]
                        nc.tensor.matmul(
                            ps,
                            w1_t[:, cc, oc * 128:(oc + 1) * 128],
                            mv,
                            start=(cc == 0), stop=(cc == CC1 - 1))
                    nc.scalar.activation(
                        out=out1p[:, h, 1 + ns * 8:1 + (ns + 1) * 8, oc, :],
                        in_=ps, func=AF.Relu, bias=b1_t[oc], scale=1.0)

        # pools for LC and later phases
        lcw_pool = ctx.enter_context(tc.tile_pool(name="lcwp", bufs=8))
        tmp_pool = ctx.enter_context(tc.tile_pool(name="tmpp", bufs=4))
        t_pool = ctx.enter_context(tc.tile_pool(name="tp", bufs=1))
        div_pool = ctx.enter_context(tc.tile_pool(name="divp", bufs=2))

        # ---- phase 2: locally-connected 3x3 + BN2 + ReLU ---------------
        # Two locations run concurrently in different PE column groups:
        # loc A -> psum partitions 0:64 (tile_position (0,0)), loc B ->
        # 64:128 ((0,64)). Patches are stationary fp16 [128, 64]; the fp8
        # weight stream is the moving operand. psum2 [128(2 locs x n), 256]
        # is copied to fp16 and PE-transposed back to [o, (2 locs x n)].
        for grp in range(NLOC // 4):
            pst = psum.tile([128, CCW, 256], FP16, name="pst", tag="tp",
                            bufs=2)
            for pair in range(2):
                locA = grp * 4 + pair * 2
                lwAB = []
                for li in range(2):
                    lw = lcw_pool.tile([128, 9, CCW, WID], FP8,
                                       name="lcw_t", tag="lcw")
                    nc.sync.dma_start(out=lw, in_=ap["lcw"][locA + li])
                    lwAB.append(lw)
                ps2 = psum.tile([128, WID], F32, name="ps2", tag="lc",
                                bufs=2)
                for kc in range(18):
                    dk, ch = divmod(kc, 2)
                    di, dj = divmod(dk, 3)
                    for li in range(2):
                        hl, j = divmod(locA + li, W)
                        nc.tensor.matmul(
                            ps2[li * 64:(li + 1) * 64, :],
                            out1p[:, hl + di, j + dj, ch, :],
                            lwAB[li][:, dk, ch, :],
                            start=(kc == 0), stop=(kc == 17),
                            tile_position=(0, li * 64))
                tmpb = tmp_pool.tile([128, 256], FP16, name="tmpb",
                                     tag="tmpb")
                nc.vector.tensor_copy(out=tmpb, in_=ps2)
                for oc in range(CCW):
                    nc.tensor.transpose(
                        pst[:, oc, pair * 128:(pair + 1) * 128],
                        tmpb[:, oc * 128:(oc + 1) * 128], ident_t)
            for oc in range(CCW):
                nc.scalar.activation(
                    out=out2_t[:, oc, grp * 256:(grp + 1) * 256],
                    in_=pst[:, oc, :], func=AF.Relu, bias=b2_t[oc],
                    scale=ISW)

        # ---- phase 3+4 merged, per 512-slice of (h,w,n): conv1x1 #2 +
        # BN3 + residual, then inhibition + divide + store. ns-outer order
        # lets slice ns start as soon as LC has produced locations
        # 8ns..8ns+7, overlapping the remaining LC weight stream.
        for ns in range(FR // 512):
            sl = slice(ns * 512, (ns + 1) * 512)
            hl, nw = divmod(ns, 2)
            tts = []
            for oc3 in range(CC3):
                ps = psum.tile([128, 512], F32, name="ps3", tag="a", bufs=3)
                for oc in range(CCW):
                    nc.tensor.matmul(
                        ps, w3_t[:, oc, oc3 * 128:(oc3 + 1) * 128],
                        out2_t[:, oc, sl],
                        start=(oc == 0), stop=(oc == CCW - 1))
                tt = t_pool.tile([128, 512], F32, name=f"tt{oc3}",
                                 tag=f"tt{oc3}")
                # t = conv3*inv3 + beta3 + x  (beta3 lives in xh)
                nc.vector.scalar_tensor_tensor(
                    out=tt, in0=ps, scalar=0.0,
                    in1=xh_t[:, oc3, hl, nw * 8:(nw + 1) * 8, :],
                    op0=ALU.add, op1=ALU.add)
                nc.scalar.activation(out=resb_t[:, oc3, sl], in_=tt,
                                     func=AF.Relu)
                tts.append(tt)
            # inhibition, low-rank: inh = F2^T (G1^T relu(t));  G1/F2 fold
            # the Gaussian mixing matrix's Fourier x Taylor factorization
            zp = psum.tile([JR, 512], F32, name="zp", tag="z", bufs=1)
            for cc in range(CC3):
                nc.tensor.matmul(zp, g1_t[:, cc, :], resb_t[:, cc, sl],
                                 start=(cc == 0), stop=(cc == CC3 - 1))
            zs = div_pool.tile([JR, 512], FP16, name="zs", tag="zs")
            nc.vector.tensor_copy(out=zs, in_=zp)
            for oc in range(CC3):
                ps = psum.tile([128, 512], F32, name="ps4", tag="a", bufs=3)
                nc.tensor.matmul(ps, f2_t[:, oc, :], zs,
                                 start=True, stop=True)
                den = div_pool.tile([128, 512], F32, name="den", tag="den")
                nc.scalar.activation(out=den, in_=ps, func=AF.Copy,
                                     scale=1.0, bias=1.0)
                rec = div_pool.tile([128, 512], F32, name="rec", tag="rec")
                nc.vector.reciprocal_approx_fast(out=rec, in_=den)
                fin = div_pool.tile([128, 512], FP16, name="fin", tag="fin")
                # final = max(t, 0) * 1/(1+inh)   (recip > 0 always)
                nc.vector.scalar_tensor_tensor(
                    out=fin, in0=tts[oc], scalar=0.0, in1=rec,
                    op0=ALU.max, op1=ALU.mult)
                nc.scalar.dma_start(out=ap["out"][oc][:, sl], in_=fin)


def _prep_inputs(x, w1, g1, b1, m1, v1, lc_w, g2, b2, m2, v2,
                 w3, g3, b3, m3, v3, sigmas):
    """Host-side shard + layout prep. Returns per-core input maps."""
    f4 = np.float32
    x = np.asarray(x, f4)
    inv1 = (g1 / np.sqrt(v1 + EPS)).astype(f4)
    beta1 = (b1 - m1 * inv1).astype(f4)
    inv2 = (g2 / np.sqrt(v2 + EPS)).astype(f4)
    beta2 = (b2 - m2 * inv2).astype(f4)
    inv3 = (g3 / np.sqrt(v3 + EPS)).astype(f4)
    beta3 = (b3 - m3 * inv3).astype(f4)

    def to8(a):
        return np.clip(a, -240.0, 240.0).astype(NPF8)

    # conv1 weight [p, cc, o] with c = cc*128 + p; bias corrected for the
    # beta3 folded into xh (conv1 must see x, not x + beta3)
    w1s = np.asarray(w1, f4) * inv1[:, None]                   # (WID, CIN)
    b1p = beta1 - w1s @ beta3
    w1t = np.ascontiguousarray(
        w1s.T.reshape(CC1, 128, WID).transpose(1, 0, 2)).astype(NPF16)
    # w3t [p, oc, o3]
    w3s = (np.asarray(w3, f4) * inv3[:, None]).T               # (WID, COUT)
    w3t = np.ascontiguousarray(
        w3s.reshape(CCW, 128, COUT).transpose(1, 0, 2)).astype(NPF16)

    # inhibition mixing matrix g[o,c] = E(d_oc; sig_c)/s(sig_c) is a
    # near-circulant Gaussian (the 1/(2.5066 sig) prefactor cancels in the
    # column normalization). Factor it as Fourier (cos series in (c-o),
    # KF modes) x Taylor (J orders in sig_c around mean sig):
    #   inh = F2^T (G1^T y),  G1[c,(j,r)] = basis_r(c) dlt_c^j/(j! s_c),
    #   F2[(j,r),o] = a_{j,k(r)} basis_r(o)
    sig = np.maximum(np.asarray(sigmas, np.float64), 0.5)
    sig0 = float(sig.mean())
    dlt = sig - sig0
    mm = np.arange(COUT)
    dm = np.abs(mm - COUT // 2).astype(np.float64)
    s = np.exp(-dm[:, None] ** 2 / (2.0 * sig[None, :] ** 2)).sum(0)
    E0 = np.exp(-dm ** 2 / (2 * sig0 ** 2))
    E1 = (dm ** 2 / sig0 ** 3) * E0
    E2 = (dm ** 4 / sig0 ** 6 - 3 * dm ** 2 / sig0 ** 4) * E0
    acoef = np.zeros((J, KF + 1))
    for j, hker in enumerate([E0, E1, E2][:J]):
        Fc = np.fft.rfft(hker).real / COUT
        a = 2.0 * Fc[:KF + 1]
        a[0] = Fc[0]
        acoef[j] = a
    ang = 2 * np.pi * mm[:, None] * np.arange(KF + 1)[None, :] / COUT
    cosb, sinb = np.cos(ang), np.sin(ang)
    basis = np.concatenate([cosb, sinb[:, 1:]], 1)             # (C, R)
    kmap = np.concatenate([np.arange(KF + 1), np.arange(1, KF + 1)])
    fact = [1.0, 1.0, 2.0, 6.0]
    G1 = np.zeros((COUT, JR))
    F2 = np.zeros((JR, COUT))
    assert J * R <= JR
    for j in range(J):
        for r in range(R):
            G1[:, j * R + r] = basis[:, r] * (dlt ** j) / (fact[j] * s)
            F2[j * R + r, :] = acoef[j, kmap[r]] * basis[:, r]
    g1t = np.ascontiguousarray(
        G1.reshape(CC3, 128, JR).transpose(1, 0, 2)).astype(NPF16)
    f2t = np.ascontiguousarray(F2.reshape(JR, CC3, 128)).astype(NPF16)

    # x (+beta3): (C, Hpad, W, N), rows zero-padded at both ends; own
    # rows shipped fp16, halo rows fp8
    xt = x.transpose(1, 2, 3, 0)                               # (C, H, W, N)
    xpad = np.zeros((CIN, H + 2, W, N), f4)
    xpad[:, 1:H + 1] = xt
    xpb = xpad + beta3[:, None, None, None]
    xh = xpb.astype(NPF16)
    xq8 = to8(xpb)

    # lc_w: (1,O,C,H,W,9) -> (H, W, p, dk, ch, o), scaled x256, fp8
    a = np.asarray(lc_w[0], f4) * (inv2[:, None, None, None, None] * SW)
    a = a.transpose(2, 3, 1, 4, 0)                 # (H, W, C, 9, O)
    a = a.reshape(H, W, CCW, 128, 9, WID).transpose(0, 1, 3, 4, 2, 5)
    lcw8 = to8(np.ascontiguousarray(a))            # (H, W, 128, 9, 2, WID)

    com = {
        "ident": np.eye(128, dtype=NPF16),
        "w1t": w1t, "w3t": w3t, "g1t": g1t, "f2t": f2t,
        "b1": b1p.reshape(CCW, 128, 1), "b2": beta2.reshape(CCW, 128, 1),
    }
    in_maps = []
    for r in range(NCORES):
        r0 = r * RPC
        xb = np.ascontiguousarray(xh[:, r0 + 1:r0 + 1 + RPC]).reshape(
            CC1, 128, RPC * W * N)
        xq = np.ascontiguousarray(xq8[:, [r0, r0 + HLO - 1]]).reshape(
            CC1, 128, 2 * W * N)
        lw = np.ascontiguousarray(lcw8[r0:r0 + RPC]).reshape(
            NLOC, 128, 9 * CCW * WID)
        if r == 0 or r == NCORES - 1:
            lw = lw.copy()
            lwv = lw.reshape(NLOC, 128, 9, CCW, WID)
            if r == 0:           # row 0 locations: di=0 taps read row -1
                lwv[0:W, :, 0:3] = 0
            if r == NCORES - 1:  # row 15 locations: di=2 taps read row 16
                lwv[W:2 * W, :, 6:9] = 0
        in_maps.append(dict(com, xh=xb, xq=xq, lcw=lw))
    return in_maps


def _assemble(results):
    """results: per-core dicts with 'out' [CC3,128,FR] -> (N,C,H,W) fp32"""
    full = np.empty((N, COUT, H, W), np.float32)
    for r, res in enumerate(results):
        o = res["out"].astype(np.float32).reshape(CC3, 128, RPC, W, N)
        # (cc, p, hl, j, n) -> (n, c, h, w)
        o = o.transpose(4, 0, 1, 2, 3).reshape(N, COUT, RPC, W)
        full[:, :, r * RPC:(r + 1) * RPC, :] = o
    return full


_NC_CACHE = {}


def get_nc(ktimes: int = 1):
    if ktimes not in _NC_CACHE:
        _NC_CACHE[ktimes] = _build_nc(ktimes)
    return _NC_CACHE[ktimes]


def kernel(**inputs):
    nc = get_nc()
    in_maps = _prep_inputs(**inputs)
    last = None
    for _ in range(3):  # rare transient device-side failures: retry
        try:
            res = run_bass_kernel_spmd(nc, in_maps,
                                       core_ids=list(range(NCORES)))
            return _assemble(res.results)
        except Exception as e:
            last = e
    raise last


if __name__ == "__main__":
    rng = np.random.default_rng(0)
    ins = {
        "x": rng.standard_normal((N, CIN, H, W), np.float32),
        "w1": (rng.standard_normal((WID, CIN), np.float32) * 0.05),
        "g1": rng.random(WID, np.float32),
        "b1": rng.standard_normal(WID, np.float32) * 0.05,
        "m1": np.zeros(WID, np.float32),
        "v1": np.ones(WID, np.float32),
        "lc_w": rng.standard_normal((1, WID, WID, H, W, 9),
                                    np.float32) * 0.05,
        "g2": rng.random(WID, np.float32),
        "b2": rng.standard_normal(WID, np.float32) * 0.05,
        "m2": np.zeros(WID, np.float32),
        "v2": np.ones(WID, np.float32),
        "w3": rng.standard_normal((COUT, WID), np.float32) * 0.05,
        "g3": rng.random(COUT, np.float32),
        "b3": rng.standard_normal(COUT, np.float32) * 0.05,
        "m3": np.zeros(COUT, np.float32),
        "v3": np.ones(COUT, np.float32),
        "sigmas": rng.random(COUT, np.float32) + COUT / 8.0,
    }
    out = kernel(**ins)
    print("out", out.shape, out.dtype, float(np.abs(out).max()))
